# revision 24
# baseline (speedup 1.0000x reference)
"""ContentStyleReltLoss kernel for 8 Trainium2 NeuronCores.

Sharding: core k handles (batch b = k//2, query-half ih = k%2).

All column-normalization / row-sum preprocessing runs on the HOST in
fp32 (exact), producing fp8e4m3 operands packed for the tensor engine's
DoubleRow perf mode (2 contraction rows per PE column per cycle):

  x^ = x/||x||  (per column), sinv' = HW / (HW - u.x^_j), u = sum_i x^_i
  lcx = x^ * sinv'_x     (content lhsT, scaled by HW to stay in fp8 range)
  lcc = -c^ * sinv'_c    (negation folded in -> single PSUM accumulation)
  ls  = s^               (style lhsT)
  rx/rc = x^/c^ query-half columns (rhs)

Device per core: for each (i-tile, j-tile) it computes
  psG = lcx^T rx + lcc^T rc  (4 DoubleRow fp8 matmuls, K=512)
  psS = ls^T rx              (2 DoubleRow fp8 matmuls, K=256)
  Scalar: |psG| summed per partition into csum_slots (content partial)
  Vector: one fused tensor_tensor_reduce: m1acc = max(m1acc, psS)
          and m2slots[:,idx] = max over i of psS  (style partials)
Final max over the 128 j-partitions of m1acc and all cross-core
combining run on the host.  content = sum|psG| / (B*HW^2);
style = max(mean(1-m1), mean(1-m2)).
"""

import numpy as np

B, C, H, W = 4, 256, 64, 64
HW = H * W          # 4096
IQ = HW // 2        # 2048 query columns per core
NCORES = 8
NJT = HW // 128     # 32 j-tiles
NIT = 2             # i-tiles
IT = IQ // NIT      # 1024

_CACHED_NC = None


def _variant():
    import os
    return os.environ.get("KVAR", "dr")  # dr | fp8 | bf16


_DVE_OP_NAME = "COPY_MAXRED_ANT"


def _dve_ref(in0, in1, s0, s1, imm2):
    b = (in0.astype(np.float32) * imm2).astype(np.float32)
    seed = (np.asarray(s1, np.float32).reshape(-1, 1)
            if isinstance(s1, np.ndarray) else s1)
    return b, np.maximum(b.reshape(b.shape[0], -1).max(axis=-1, keepdims=True),
                         seed)


def _register_copy_maxred():
    """Fused DVE ucode op: out = in0 (dtype-cast copy), accum_out =
    max(s1, max over free dim of in0). One DVE pass gives both the
    bf16 staging copy of psS and the clean per-partition m2 max."""
    from concourse import dve_ops as D
    for o in D.OPS:
        if o.name == _DVE_OP_NAME:
            return o
    from concourse.dve_spec import Spec, Src0, C1, C2, maxx, lower
    from concourse.dve_uop import DveOpSpec
    spec = Spec(body=Src0 * C2, accum=maxx, accum_init=C1, reference=_dve_ref)
    row = D._CUSTOM_DVE_ROW_BASE + len(D.OPS)
    assert row < 0x20
    D._SUB_OPCODE_FOR_NAME[_DVE_OP_NAME] = row
    ver = "v3"
    compiled = DveOpSpec(name=_DVE_OP_NAME, opcode=row,
                         uops=lower(spec, ver=ver), rd1_en=False)
    op = D.DveOp(_DVE_OP_NAME, spec, False,
                 uops_sha={ver: compiled.sha(ver)})
    D.OPS.append(op)
    D.CUSTOM_DVE_SPECS[_DVE_OP_NAME] = spec
    return op


def _build(repeat=1):
    import concourse.bacc as bacc
    import concourse.tile as tile
    from concourse import mybir
    from concourse.alu_op_type import AluOpType
    from contextlib import ExitStack

    dt = mybir.dt
    AF = mybir.ActivationFunctionType
    AX = mybir.AxisListType
    PM = mybir.MatmulPerfMode
    var = _variant()
    idt = dt.bfloat16 if var == "bf16" else dt.float8e4

    nc = bacc.Bacc(None)

    lcx = nc.dram_tensor("lcx", [128, 2, HW], idt, kind="ExternalInput")
    lcc = nc.dram_tensor("lcc", [128, 2, HW], idt, kind="ExternalInput")
    ls = nc.dram_tensor("ls", [128, 2, HW], idt, kind="ExternalInput")
    rx = nc.dram_tensor("rx", [128, 2, IQ], idt, kind="ExternalInput")
    rc = nc.dram_tensor("rc", [128, 2, IQ], idt, kind="ExternalInput")

    o_cs = nc.dram_tensor("csum", [128, NJT * NIT], dt.float32,
                          kind="ExternalOutput")
    o_m1 = nc.dram_tensor("m1acc", [128, IQ], dt.bfloat16,
                          kind="ExternalOutput")
    o_m2 = nc.dram_tensor("m2part", [128, NJT], dt.float32,
                          kind="ExternalOutput")

    with tile.TileContext(nc) as tc, ExitStack() as top:
        pers = top.enter_context(tc.tile_pool(name="pers", bufs=1))
        for _rep in range(repeat):
            LCX = pers.tile([128, 2, HW], idt, tag="lcx", name="LCX")
            LCC = pers.tile([128, 2, HW], idt, tag="lcc", name="LCC")
            LS = pers.tile([128, 2, HW], idt, tag="ls", name="LS")
            RX = pers.tile([128, 2, IQ], idt, tag="rx", name="RX")
            RC = pers.tile([128, 2, IQ], idt, tag="rc", name="RC")
            csum_slots = pers.tile([128, NJT * NIT], dt.float32, tag="cslot",
                                   name="cslot")
            m2slots = pers.tile([128, NJT], dt.float32, tag="m2slot",
                                name="m2slot")
            m1acc = pers.tile([128, IQ], dt.bfloat16, tag="m1acc", name="m1acc")

            nc.gpsimd.memset(m1acc[:], -2.0)
            # Loads split across both hwdge queues (sync + scalar), ordered
            # first-needed-first so jt=0/it=0 tiles start ~4us in.
            HH = HW // 2
            IH = IQ // 2
            for h in range(2):
                hs = slice(h * HH, (h + 1) * HH)
                qs = slice(h * IH, (h + 1) * IH)
                nc.sync.dma_start(LCX[:, :, hs], lcx[:, :, hs])
                nc.scalar.dma_start(LCC[:, :, hs], lcc[:, :, hs])
                nc.sync.dma_start(RX[:, :, qs], rx[:, :, qs])
                nc.scalar.dma_start(RC[:, :, qs], rc[:, :, qs])
                nc.scalar.dma_start(LS[:, :, hs], ls[:, :, hs])

            def mm(out, lhsT3, rhs3, js, ms, start, stop):
                """DoubleRow fp8, or plain per-row-pair matmuls."""
                if var == "dr":
                    nc.tensor.matmul(out, lhsT3[:, :, js], rhs3[:, :, ms],
                                     start=start, stop=stop,
                                     perf_mode=PM.DoubleRow)
                else:
                    nc.tensor.matmul(out, lhsT3[:, 0, js], rhs3[:, 0, ms],
                                     start=start, stop=False)
                    nc.tensor.matmul(out, lhsT3[:, 1, js], rhs3[:, 1, ms],
                                     start=False, stop=stop)

            dve_op = _register_copy_maxred()

            with tc.tile_pool(name="cps", bufs=2, space="PSUM") as cps, \
                 tc.tile_pool(name="sps", bufs=1, space="PSUM") as sps, \
                 tc.tile_pool(name="dmp", bufs=2) as dmp:
                for jt in range(NJT):
                    js = slice(jt * 128, (jt + 1) * 128)
                    # content: psG = lcx^T rx + lcc^T rc   (K = 512)
                    for it in range(NIT):
                        idx = jt * NIT + it
                        psG = cps.tile([128, IT], dt.float32, tag="psG",
                                       name=f"psG{idx}")
                        for m in range(2):
                            ms = slice(it * IT + m * 512, it * IT + (m + 1) * 512)
                            os = slice(m * 512, (m + 1) * 512)
                            mm(psG[:, os], LCX, RX, js, ms, True, False)
                            mm(psG[:, os], LCC, RC, js, ms, False, True)
                        dump = dmp.tile([128, IT], dt.bfloat16, tag="adump",
                                        name=f"adump{idx}")
                        nc.scalar.activation(
                            dump[:], psG[:], AF.Abs,
                            accum_out=csum_slots[:, idx:idx + 1])
                    # style: psS = ls^T rx over the FULL i-range (K = 256)
                    psS = sps.tile([128, IQ], dt.float32, tag="psS",
                                   name=f"psS{jt}")
                    for m in range(4):
                        ms = slice(m * 512, (m + 1) * 512)
                        mm(psS[:, ms], LS, RX, js, ms, True, True)
                    # ONE fused DVE pass: bf16 staging copy + clean per-j m2
                    # (max over all 2048 i of this core), then one full-width
                    # 2x-mode bf16 max into the m1 running accumulator.
                    stage = dmp.tile([128, IQ], dt.bfloat16, tag="stg",
                                     name=f"stg{jt}")
                    nc.vector._custom_dve(
                        dve_op, out=stage[:], in0=psS[:], s1=-2.0,
                        imm2=1.0, accum_out=m2slots[:, jt:jt + 1])
                    nc.vector.tensor_max(m1acc[:], m1acc[:], stage[:])
                nc.sync.dma_start(o_m1[:], m1acc[:])
                nc.sync.dma_start(o_cs[:], csum_slots[:])
                nc.sync.dma_start(o_m2[:], m2slots[:])

    nc.finalize()
    return nc


def _get_nc():
    global _CACHED_NC
    if _CACHED_NC is None:
        import os
        _CACHED_NC = _build(repeat=int(os.environ.get("KREPEAT", "1")))
    return _CACHED_NC


_RUNNER = None


def _get_runner():
    """Compile the 8-core PJRT executable once; returns run(in_maps)->results.

    Mirrors concourse.bass2jax.run_bass_via_pjrt but caches the jitted
    executable so repeated kernel() calls only pay device execution.
    """
    global _RUNNER
    if _RUNNER is not None:
        return _RUNNER
    import jax
    import numpy as _np
    from jax.sharding import Mesh, PartitionSpec
    from jax.experimental.shard_map import shard_map
    from concourse import mybir, bass2jax
    from concourse.bass2jax import _bass_exec_p, partition_id_tensor

    bass2jax.install_neuronx_cc_hook()
    nc = _get_nc()
    partition_name = (nc.partition_id_tensor.name
                      if nc.partition_id_tensor else None)

    in_names, out_names, out_avals, zero_outs = [], [], [], []
    for alloc in nc.m.functions[0].allocations:
        if not isinstance(alloc, mybir.MemoryLocationSet):
            continue
        name = alloc.memorylocations[0].name
        if alloc.kind == "ExternalInput":
            if name != partition_name:
                in_names.append(name)
        elif alloc.kind == "ExternalOutput":
            out_names.append(name)
            shape = tuple(alloc.tensor_shape)
            dtype = mybir.dt.np(alloc.dtype)
            out_avals.append(jax.core.ShapedArray(shape, dtype))
            zero_outs.append(_np.zeros((NCORES * shape[0], *shape[1:]), dtype))
    n_params = len(in_names)
    n_outs = len(out_avals)
    all_names = list(in_names) + list(out_names)
    if partition_name is not None:
        all_names.append(partition_name)
    donate = tuple(range(n_params, n_params + n_outs))

    def _body(*args):
        operands = list(args)
        if partition_name is not None:
            operands.append(partition_id_tensor())
        return tuple(_bass_exec_p.bind(
            *operands,
            out_avals=tuple(out_avals),
            in_names=tuple(all_names),
            out_names=tuple(out_names),
            lowering_input_output_aliases=(),
            sim_require_finite=True,
            sim_require_nnan=True,
            nc=nc,
        ))

    devices = jax.devices()[:NCORES]
    mesh = Mesh(_np.asarray(devices), ("core",))
    sharded = jax.jit(
        shard_map(_body, mesh=mesh,
                  in_specs=(PartitionSpec("core"),) * (n_params + n_outs),
                  out_specs=(PartitionSpec("core"),) * n_outs,
                  check_rep=False),
        donate_argnums=donate, keep_unused=True,
    )

    def prepare(in_maps):
        """Stage concatenated inputs onto the devices once (for timing)."""
        from jax.sharding import NamedSharding
        sh = NamedSharding(mesh, PartitionSpec("core"))
        concat_in = [
            _np.concatenate([in_maps[c][nm] for c in range(NCORES)], axis=0)
            for nm in in_names
        ]
        return [jax.device_put(a, sh) for a in concat_in]

    def exec_prepared(staged):
        out_arrs = sharded(*staged, *zero_outs)
        jax.block_until_ready(out_arrs)
        return out_arrs

    def run(in_maps):
        concat_in = [
            _np.concatenate([in_maps[c][nm] for c in range(NCORES)], axis=0)
            for nm in in_names
        ]
        out_arrs = sharded(*concat_in, *zero_outs)
        jax.block_until_ready(out_arrs)
        return [
            {nm: _np.asarray(out_arrs[i]).reshape(NCORES, *out_avals[i].shape)[c]
             for i, nm in enumerate(out_names)}
            for c in range(NCORES)
        ]

    run.prepare = prepare
    run.exec_prepared = exec_prepared
    _RUNNER = run
    return run


def _pack(a):
    """[C, N] -> DoubleRow layout [128, 2, N]: channel c = r*128 + k."""
    n = a.shape[1]
    return np.ascontiguousarray(a.reshape(2, 128, n).transpose(1, 0, 2))


def _make_in_maps(x_feat, c_feat, s_feat):
    from concourse import mybir
    f8 = mybir.dt.np(mybir.dt.bfloat16 if _variant() == "bf16"
                     else mybir.dt.float8e4)
    x = np.asarray(x_feat, dtype=np.float32).reshape(B, C, HW)
    c = np.asarray(c_feat, dtype=np.float32).reshape(B, C, HW)
    s = np.asarray(s_feat, dtype=np.float32).reshape(B, C, HW)

    in_maps = [dict() for _ in range(NCORES)]
    for b in range(B):
        def hat(t):
            return t / np.sqrt((t * t).sum(axis=0, keepdims=True))
        hx, hc, hs = hat(x[b]), hat(c[b]), hat(s[b])

        def sinv_scaled(h):
            u = h.sum(axis=1)
            su = u @ h
            return HW / (HW - su)
        lx = _pack((hx * sinv_scaled(hx)[None, :]).astype(f8))
        lc = _pack((-hc * sinv_scaled(hc)[None, :]).astype(f8))
        lsb = _pack(hs.astype(f8))
        rxb = _pack(hx.astype(f8))
        rcb = _pack(hc.astype(f8))
        for ih in range(2):
            sl = slice(ih * IQ, (ih + 1) * IQ)
            in_maps[2 * b + ih] = {
                "lcx": lx, "lcc": lc, "ls": lsb,
                "rx": np.ascontiguousarray(rxb[:, :, sl]),
                "rc": np.ascontiguousarray(rcb[:, :, sl]),
            }
    return in_maps


def kernel(x_feat, c_feat, s_feat):
    outs = _get_runner()(_make_in_maps(x_feat, c_feat, s_feat))

    total = sum(float(r["csum"].sum()) for r in outs)
    content = total / (B * HW * HW)

    m1vals = 1.0 - np.concatenate(
        [r["m1acc"].astype(np.float32).max(axis=0) for r in outs])
    m1mean = float(m1vals.mean())
    m2mean = 0.0
    for b_ in range(B):
        mx = np.maximum(outs[2 * b_]["m2part"], outs[2 * b_ + 1]["m2part"])
        m2mean += float((1.0 - mx).mean())
    m2mean /= B
    style = max(m1mean, m2mean)

    return (np.float32(content), np.float32(style))


# revision 30
# speedup vs baseline: 1.0943x; 1.0943x over previous
"""ContentStyleReltLoss kernel for 8 Trainium2 NeuronCores.

Sharding: core k handles (batch b = k//2, query-half ih = k%2).

All column-normalization / row-sum preprocessing runs on the HOST in
fp32 (exact), producing fp8e4m3 operands packed for the tensor engine's
DoubleRow perf mode (2 contraction rows per PE column per cycle):

  x^ = x/||x||  (per column), sinv' = HW / (HW - u.x^_j), u = sum_i x^_i
  lcx = x^ * sinv'_x     (content lhsT, scaled by HW to stay in fp8 range)
  lcc = -c^ * sinv'_c    (negation folded in -> single PSUM accumulation)
  ls  = s^               (style lhsT)
  rx/rc = x^/c^ query-half columns (rhs)

Device per core: for each (i-tile, j-tile) it computes
  psG = lcx^T rx + lcc^T rc  (4 DoubleRow fp8 matmuls, K=512)
  psS = ls^T rx              (2 DoubleRow fp8 matmuls, K=256)
  Scalar: |psG| summed per partition into csum_slots (content partial)
  Vector: one fused tensor_tensor_reduce: m1acc = max(m1acc, psS)
          and m2slots[:,idx] = max over i of psS  (style partials)
Final max over the 128 j-partitions of m1acc and all cross-core
combining run on the host.  content = sum|psG| / (B*HW^2);
style = max(mean(1-m1), mean(1-m2)).
"""

import numpy as np

B, C, H, W = 4, 256, 64, 64
HW = H * W          # 4096
IQ = HW // 2        # 2048 query columns per core
NCORES = 8
NJT = HW // 128     # 32 j-tiles
NIT = 2             # i-tiles
IT = IQ // NIT      # 1024

_CACHED_NC = None


def _variant():
    import os
    return os.environ.get("KVAR", "dr")  # dr | fp8 | bf16


_DVE_OP_NAME = "COPY_MAXRED_ANT"


def _dve_ref(in0, in1, s0, s1, imm2):
    b = (in0.astype(np.float32) * imm2).astype(np.float32)
    seed = (np.asarray(s1, np.float32).reshape(-1, 1)
            if isinstance(s1, np.ndarray) else s1)
    return b, np.maximum(b.reshape(b.shape[0], -1).max(axis=-1, keepdims=True),
                         seed)


def _register_copy_maxred():
    """Fused DVE ucode op: out = in0 (dtype-cast copy), accum_out =
    max(s1, max over free dim of in0). One DVE pass gives both the
    bf16 staging copy of psS and the clean per-partition m2 max."""
    from concourse import dve_ops as D
    for o in D.OPS:
        if o.name == _DVE_OP_NAME:
            return o
    from concourse.dve_spec import Spec, Src0, C1, C2, maxx, lower
    from concourse.dve_uop import DveOpSpec
    spec = Spec(body=Src0 * C2, accum=maxx, accum_init=C1, reference=_dve_ref)
    row = D._CUSTOM_DVE_ROW_BASE + len(D.OPS)
    assert row < 0x20
    D._SUB_OPCODE_FOR_NAME[_DVE_OP_NAME] = row
    ver = "v3"
    compiled = DveOpSpec(name=_DVE_OP_NAME, opcode=row,
                         uops=lower(spec, ver=ver), rd1_en=False)
    op = D.DveOp(_DVE_OP_NAME, spec, False,
                 uops_sha={ver: compiled.sha(ver)})
    D.OPS.append(op)
    D.CUSTOM_DVE_SPECS[_DVE_OP_NAME] = spec
    return op


def _build(repeat=1):
    import concourse.bacc as bacc
    import concourse.tile as tile
    from concourse import mybir
    from concourse.alu_op_type import AluOpType
    from contextlib import ExitStack

    dt = mybir.dt
    AF = mybir.ActivationFunctionType
    AX = mybir.AxisListType
    PM = mybir.MatmulPerfMode
    var = _variant()
    idt = dt.bfloat16 if var == "bf16" else dt.float8e4

    nc = bacc.Bacc(None)

    lcx = nc.dram_tensor("lcx", [128, 2, HW], idt, kind="ExternalInput")
    lcc = nc.dram_tensor("lcc", [128, 2, HW], idt, kind="ExternalInput")
    ls = nc.dram_tensor("ls", [128, 2, HW], idt, kind="ExternalInput")
    rx = nc.dram_tensor("rx", [128, 2, IQ], idt, kind="ExternalInput")
    rc = nc.dram_tensor("rc", [128, 2, IQ], idt, kind="ExternalInput")

    o_cs = nc.dram_tensor("csum", [128, NJT * NIT], dt.float32,
                          kind="ExternalOutput")
    o_m1 = nc.dram_tensor("m1acc", [128, IQ], dt.bfloat16,
                          kind="ExternalOutput")
    o_m2 = nc.dram_tensor("m2part", [128, NJT * NIT], dt.float32,
                          kind="ExternalOutput")

    with tile.TileContext(nc) as tc, ExitStack() as top:
        pers = top.enter_context(tc.tile_pool(name="pers", bufs=1))
        for _rep in range(repeat):
            LCX = pers.tile([128, 2, HW], idt, tag="lcx", name="LCX")
            LCC = pers.tile([128, 2, HW], idt, tag="lcc", name="LCC")
            LS = pers.tile([128, 2, HW], idt, tag="ls", name="LS")
            RX = pers.tile([128, 2, IQ], idt, tag="rx", name="RX")
            RC = pers.tile([128, 2, IQ], idt, tag="rc", name="RC")
            csum_slots = pers.tile([128, NJT * NIT], dt.float32, tag="cslot",
                                   name="cslot")
            m2slots = pers.tile([128, NJT * NIT], dt.float32, tag="m2slot",
                                name="m2slot")
            m1acc = pers.tile([128, IQ], dt.bfloat16, tag="m1acc", name="m1acc")

            nc.gpsimd.memset(m1acc[:], -2.0)
            # Loads split across both hwdge queues (sync + scalar), ordered
            # first-needed-first. The leading chunks are quartered so the
            # first j-tiles are fed ~2-3us after the preamble.
            HQ = HW // 4
            IH = IQ // 2
            for h in range(4):
                hs = slice(h * HQ, (h + 1) * HQ)
                nc.sync.dma_start(LCX[:, :, hs], lcx[:, :, hs])
                nc.scalar.dma_start(LCC[:, :, hs], lcc[:, :, hs])
                if h < 2:
                    qs = slice(h * IH, (h + 1) * IH)
                    nc.sync.dma_start(RX[:, :, qs], rx[:, :, qs])
                    nc.scalar.dma_start(RC[:, :, qs], rc[:, :, qs])
                nc.scalar.dma_start(LS[:, :, hs], ls[:, :, hs])

            def mm(out, lhsT3, rhs3, js, ms, start, stop):
                """DoubleRow fp8, or plain per-row-pair matmuls."""
                if var == "dr":
                    nc.tensor.matmul(out, lhsT3[:, :, js], rhs3[:, :, ms],
                                     start=start, stop=stop,
                                     perf_mode=PM.DoubleRow)
                else:
                    nc.tensor.matmul(out, lhsT3[:, 0, js], rhs3[:, 0, ms],
                                     start=start, stop=False)
                    nc.tensor.matmul(out, lhsT3[:, 1, js], rhs3[:, 1, ms],
                                     start=False, stop=stop)

            dve_op = _register_copy_maxred()
            import os as _os
            nsc = int(_os.environ.get("NSC", "0"))  # S-copied tiles per 32

            with tc.tile_pool(name="cps", bufs=2, space="PSUM") as cps, \
                 tc.tile_pool(name="sps", bufs=2, space="PSUM") as sps, \
                 tc.tile_pool(name="dmp", bufs=2) as dmp:
                for it in range(NIT):
                    sl = m1acc[:, it * IT:(it + 1) * IT]
                    for jt in range(NJT):
                        js = slice(jt * 128, (jt + 1) * 128)
                        idx = jt * NIT + it
                        scopy = (jt * nsc) // NJT != ((jt + 1) * nsc) // NJT
                        # content: psG = lcx^T rx + lcc^T rc   (K = 512)
                        psG = cps.tile([128, IT], dt.float32, tag="psG",
                                       name=f"psG{idx}")
                        for m in range(2):
                            ms = slice(it * IT + m * 512, it * IT + (m + 1) * 512)
                            os = slice(m * 512, (m + 1) * 512)
                            mm(psG[:, os], LCX, RX, js, ms, True, False)
                            mm(psG[:, os], LCC, RC, js, ms, False, True)
                        # style: psS = ls^T rx   (K = 256)
                        psS = sps.tile([128, IT], dt.float32, tag="psS",
                                       name=f"psS{idx}")
                        for m in range(2):
                            ms = slice(it * IT + m * 512, it * IT + (m + 1) * 512)
                            os = slice(m * 512, (m + 1) * 512)
                            mm(psS[:, os], LS, RX, js, ms, True, True)
                        stage = dmp.tile([128, IT], dt.bfloat16, tag="stg",
                                         name=f"stg{idx}")
                        if scopy:
                            # offload the PSUM->bf16 staging copy to Scalar
                            # (issued before the abs so the DVE isn't blocked)
                            nc.scalar.activation(stage[:], psS[:], AF.Copy)
                        # content |.| accumulate on Scalar
                        dump = dmp.tile([128, IT], dt.bfloat16, tag="adump",
                                        name=f"adump{idx}")
                        nc.scalar.activation(
                            dump[:], psG[:], AF.Abs,
                            accum_out=csum_slots[:, idx:idx + 1])
                        # style: ONE fused DVE pass = bf16 staging copy of
                        # psS + clean per-j m2 max; then a cheap 2x-mode
                        # bf16 max folds the stage into the m1 running max.
                        if scopy:
                            nc.vector.reduce_max(m2slots[:, idx:idx + 1],
                                                 stage[:], axis=AX.X)
                        else:
                            nc.vector._custom_dve(
                                dve_op, out=stage[:], in0=psS[:], s1=-2.0,
                                imm2=1.0,
                                accum_out=m2slots[:, idx:idx + 1])
                        nc.vector.tensor_max(sl, sl, stage[:])
                    # stream out the finished half of m1acc
                    nc.sync.dma_start(o_m1[:, it * IT:(it + 1) * IT],
                                      m1acc[:, it * IT:(it + 1) * IT])
                nc.sync.dma_start(o_cs[:], csum_slots[:])
                nc.sync.dma_start(o_m2[:], m2slots[:])

    nc.finalize()
    return nc


def _get_nc():
    global _CACHED_NC
    if _CACHED_NC is None:
        import os
        _CACHED_NC = _build(repeat=int(os.environ.get("KREPEAT", "1")))
    return _CACHED_NC


_RUNNER = None


def _get_runner():
    """Compile the 8-core PJRT executable once; returns run(in_maps)->results.

    Mirrors concourse.bass2jax.run_bass_via_pjrt but caches the jitted
    executable so repeated kernel() calls only pay device execution.
    """
    global _RUNNER
    if _RUNNER is not None:
        return _RUNNER
    import jax
    import numpy as _np
    from jax.sharding import Mesh, PartitionSpec
    from jax.experimental.shard_map import shard_map
    from concourse import mybir, bass2jax
    from concourse.bass2jax import _bass_exec_p, partition_id_tensor

    bass2jax.install_neuronx_cc_hook()
    nc = _get_nc()
    partition_name = (nc.partition_id_tensor.name
                      if nc.partition_id_tensor else None)

    in_names, out_names, out_avals, zero_outs = [], [], [], []
    for alloc in nc.m.functions[0].allocations:
        if not isinstance(alloc, mybir.MemoryLocationSet):
            continue
        name = alloc.memorylocations[0].name
        if alloc.kind == "ExternalInput":
            if name != partition_name:
                in_names.append(name)
        elif alloc.kind == "ExternalOutput":
            out_names.append(name)
            shape = tuple(alloc.tensor_shape)
            dtype = mybir.dt.np(alloc.dtype)
            out_avals.append(jax.core.ShapedArray(shape, dtype))
            zero_outs.append(_np.zeros((NCORES * shape[0], *shape[1:]), dtype))
    n_params = len(in_names)
    n_outs = len(out_avals)
    all_names = list(in_names) + list(out_names)
    if partition_name is not None:
        all_names.append(partition_name)
    donate = tuple(range(n_params, n_params + n_outs))

    def _body(*args):
        operands = list(args)
        if partition_name is not None:
            operands.append(partition_id_tensor())
        return tuple(_bass_exec_p.bind(
            *operands,
            out_avals=tuple(out_avals),
            in_names=tuple(all_names),
            out_names=tuple(out_names),
            lowering_input_output_aliases=(),
            sim_require_finite=True,
            sim_require_nnan=True,
            nc=nc,
        ))

    devices = jax.devices()[:NCORES]
    mesh = Mesh(_np.asarray(devices), ("core",))
    sharded = jax.jit(
        shard_map(_body, mesh=mesh,
                  in_specs=(PartitionSpec("core"),) * (n_params + n_outs),
                  out_specs=(PartitionSpec("core"),) * n_outs,
                  check_rep=False),
        donate_argnums=donate, keep_unused=True,
    )

    def prepare(in_maps):
        """Stage concatenated inputs onto the devices once (for timing)."""
        from jax.sharding import NamedSharding
        sh = NamedSharding(mesh, PartitionSpec("core"))
        concat_in = [
            _np.concatenate([in_maps[c][nm] for c in range(NCORES)], axis=0)
            for nm in in_names
        ]
        return [jax.device_put(a, sh) for a in concat_in]

    def exec_prepared(staged):
        out_arrs = sharded(*staged, *zero_outs)
        jax.block_until_ready(out_arrs)
        return out_arrs

    def run(in_maps):
        concat_in = [
            _np.concatenate([in_maps[c][nm] for c in range(NCORES)], axis=0)
            for nm in in_names
        ]
        out_arrs = sharded(*concat_in, *zero_outs)
        jax.block_until_ready(out_arrs)
        return [
            {nm: _np.asarray(out_arrs[i]).reshape(NCORES, *out_avals[i].shape)[c]
             for i, nm in enumerate(out_names)}
            for c in range(NCORES)
        ]

    run.prepare = prepare
    run.exec_prepared = exec_prepared
    _RUNNER = run
    return run


def _pack(a):
    """[C, N] -> DoubleRow layout [128, 2, N]: channel c = r*128 + k."""
    n = a.shape[1]
    return np.ascontiguousarray(a.reshape(2, 128, n).transpose(1, 0, 2))


def _make_in_maps(x_feat, c_feat, s_feat):
    from concourse import mybir
    f8 = mybir.dt.np(mybir.dt.bfloat16 if _variant() == "bf16"
                     else mybir.dt.float8e4)
    x = np.asarray(x_feat, dtype=np.float32).reshape(B, C, HW)
    c = np.asarray(c_feat, dtype=np.float32).reshape(B, C, HW)
    s = np.asarray(s_feat, dtype=np.float32).reshape(B, C, HW)

    in_maps = [dict() for _ in range(NCORES)]
    for b in range(B):
        def hat(t):
            return t / np.sqrt((t * t).sum(axis=0, keepdims=True))
        hx, hc, hs = hat(x[b]), hat(c[b]), hat(s[b])

        def sinv_scaled(h):
            u = h.sum(axis=1)
            su = u @ h
            return HW / (HW - su)
        lx = _pack((hx * sinv_scaled(hx)[None, :]).astype(f8))
        lc = _pack((-hc * sinv_scaled(hc)[None, :]).astype(f8))
        lsb = _pack(hs.astype(f8))
        rxb = _pack(hx.astype(f8))
        rcb = _pack(hc.astype(f8))
        for ih in range(2):
            sl = slice(ih * IQ, (ih + 1) * IQ)
            in_maps[2 * b + ih] = {
                "lcx": lx, "lcc": lc, "ls": lsb,
                "rx": np.ascontiguousarray(rxb[:, :, sl]),
                "rc": np.ascontiguousarray(rcb[:, :, sl]),
            }
    return in_maps


def kernel(x_feat, c_feat, s_feat):
    outs = _get_runner()(_make_in_maps(x_feat, c_feat, s_feat))

    total = sum(float(r["csum"].sum()) for r in outs)
    content = total / (B * HW * HW)

    m1vals = 1.0 - np.concatenate(
        [r["m1acc"].astype(np.float32).max(axis=0) for r in outs])
    m1mean = float(m1vals.mean())
    m2mean = 0.0
    for b_ in range(B):
        m2a = outs[2 * b_]["m2part"].reshape(128, NJT, NIT).max(axis=2)
        m2b = outs[2 * b_ + 1]["m2part"].reshape(128, NJT, NIT).max(axis=2)
        mx = np.maximum(m2a, m2b)
        m2mean += float((1.0 - mx).mean())
    m2mean /= B
    style = max(m1mean, m2mean)

    return (np.float32(content), np.float32(style))


# revision 31
# speedup vs baseline: 1.1223x; 1.0256x over previous
"""ContentStyleReltLoss kernel for 8 Trainium2 NeuronCores.

Sharding: core k handles (batch b = k//2, query-half ih = k%2).

All column-normalization / row-sum preprocessing runs on the HOST in
fp32 (exact), producing fp8e4m3 operands packed for the tensor engine's
DoubleRow perf mode (2 contraction rows per PE column per cycle):

  x^ = x/||x||  (per column), sinv' = HW / (HW - u.x^_j), u = sum_i x^_i
  lcx = x^ * sinv'_x     (content lhsT, scaled by HW to stay in fp8 range)
  lcc = -c^ * sinv'_c    (negation folded in -> single PSUM accumulation)
  ls  = s^               (style lhsT)
  rx/rc = x^/c^ query-half columns (rhs)

Device per core: for each (i-tile, j-tile) it computes
  psG = lcx^T rx + lcc^T rc  (4 DoubleRow fp8 matmuls, K=512)
  psS = ls^T rx              (2 DoubleRow fp8 matmuls, K=256)
  Scalar: |psG| summed per partition into csum_slots (content partial)
  Vector: one fused tensor_tensor_reduce: m1acc = max(m1acc, psS)
          and m2slots[:,idx] = max over i of psS  (style partials)
Final max over the 128 j-partitions of m1acc and all cross-core
combining run on the host.  content = sum|psG| / (B*HW^2);
style = max(mean(1-m1), mean(1-m2)).
"""

import numpy as np

B, C, H, W = 4, 256, 64, 64
HW = H * W          # 4096
IQ = HW // 2        # 2048 query columns per core
NCORES = 8
NJT = HW // 128     # 32 j-tiles
NIT = 2             # i-tiles
IT = IQ // NIT      # 1024

_CACHED_NC = None


def _variant():
    import os
    return os.environ.get("KVAR", "dr")  # dr | fp8 | bf16


_DVE_OP_NAME = "COPY_MAXRED_ANT"


def _dve_ref(in0, in1, s0, s1, imm2):
    b = (in0.astype(np.float32) * imm2).astype(np.float32)
    seed = (np.asarray(s1, np.float32).reshape(-1, 1)
            if isinstance(s1, np.ndarray) else s1)
    return b, np.maximum(b.reshape(b.shape[0], -1).max(axis=-1, keepdims=True),
                         seed)


def _register_copy_maxred():
    """Fused DVE ucode op: out = in0 (dtype-cast copy), accum_out =
    max(s1, max over free dim of in0). One DVE pass gives both the
    bf16 staging copy of psS and the clean per-partition m2 max."""
    from concourse import dve_ops as D
    for o in D.OPS:
        if o.name == _DVE_OP_NAME:
            return o
    from concourse.dve_spec import Spec, Src0, C1, C2, maxx, lower
    from concourse.dve_uop import DveOpSpec
    spec = Spec(body=Src0 * C2, accum=maxx, accum_init=C1, reference=_dve_ref)
    row = D._CUSTOM_DVE_ROW_BASE + len(D.OPS)
    assert row < 0x20
    D._SUB_OPCODE_FOR_NAME[_DVE_OP_NAME] = row
    ver = "v3"
    compiled = DveOpSpec(name=_DVE_OP_NAME, opcode=row,
                         uops=lower(spec, ver=ver), rd1_en=False)
    op = D.DveOp(_DVE_OP_NAME, spec, False,
                 uops_sha={ver: compiled.sha(ver)})
    D.OPS.append(op)
    D.CUSTOM_DVE_SPECS[_DVE_OP_NAME] = spec
    return op


def _build(repeat=1):
    import concourse.bacc as bacc
    import concourse.tile as tile
    from concourse import mybir
    from concourse.alu_op_type import AluOpType
    from contextlib import ExitStack

    dt = mybir.dt
    AF = mybir.ActivationFunctionType
    AX = mybir.AxisListType
    PM = mybir.MatmulPerfMode
    var = _variant()
    idt = dt.bfloat16 if var == "bf16" else dt.float8e4

    nc = bacc.Bacc(None)

    lcx = nc.dram_tensor("lcx", [128, 2, HW], idt, kind="ExternalInput")
    lcc = nc.dram_tensor("lcc", [128, 2, HW], idt, kind="ExternalInput")
    ls = nc.dram_tensor("ls", [128, 2, HW], idt, kind="ExternalInput")
    rx = nc.dram_tensor("rx", [128, 2, IQ], idt, kind="ExternalInput")
    rc = nc.dram_tensor("rc", [128, 2, IQ], idt, kind="ExternalInput")

    o_cs = nc.dram_tensor("csum", [128, NJT * NIT], dt.float32,
                          kind="ExternalOutput")
    o_m1 = nc.dram_tensor("m1acc", [128, IQ], dt.bfloat16,
                          kind="ExternalOutput")
    o_m2 = nc.dram_tensor("m2part", [128, NJT * NIT], dt.float32,
                          kind="ExternalOutput")

    with tile.TileContext(nc) as tc, ExitStack() as top:
        pers = top.enter_context(tc.tile_pool(name="pers", bufs=1))
        for _rep in range(repeat):
            LCX = pers.tile([128, 2, HW], idt, tag="lcx", name="LCX")
            LCC = pers.tile([128, 2, HW], idt, tag="lcc", name="LCC")
            LS = pers.tile([128, 2, HW], idt, tag="ls", name="LS")
            RX = pers.tile([128, 2, IQ], idt, tag="rx", name="RX")
            RC = pers.tile([128, 2, IQ], idt, tag="rc", name="RC")
            csum_slots = pers.tile([128, NJT * NIT], dt.float32, tag="cslot",
                                   name="cslot")
            m2slots = pers.tile([128, NJT * NIT], dt.float32, tag="m2slot",
                                name="m2slot")
            m1acc = pers.tile([128, IQ], dt.bfloat16, tag="m1acc", name="m1acc")

            nc.gpsimd.memset(m1acc[:], -2.0)
            # Loads split across both hwdge queues (sync + scalar), ordered
            # first-needed-first so jt=0/it=0 tiles start ~4us in.
            HH = HW // 2
            IH = IQ // 2
            for h in range(2):
                hs = slice(h * HH, (h + 1) * HH)
                qs = slice(h * IH, (h + 1) * IH)
                nc.sync.dma_start(LCX[:, :, hs], lcx[:, :, hs])
                nc.scalar.dma_start(LCC[:, :, hs], lcc[:, :, hs])
                nc.sync.dma_start(RX[:, :, qs], rx[:, :, qs])
                nc.scalar.dma_start(RC[:, :, qs], rc[:, :, qs])
                nc.scalar.dma_start(LS[:, :, hs], ls[:, :, hs])

            def mm(out, lhsT3, rhs3, js, ms, start, stop):
                """DoubleRow fp8, or plain per-row-pair matmuls."""
                if var == "dr":
                    nc.tensor.matmul(out, lhsT3[:, :, js], rhs3[:, :, ms],
                                     start=start, stop=stop,
                                     perf_mode=PM.DoubleRow)
                else:
                    nc.tensor.matmul(out, lhsT3[:, 0, js], rhs3[:, 0, ms],
                                     start=start, stop=False)
                    nc.tensor.matmul(out, lhsT3[:, 1, js], rhs3[:, 1, ms],
                                     start=False, stop=stop)

            dve_op = _register_copy_maxred()
            import os as _os
            nsc = int(_os.environ.get("NSC", "0"))  # S-copied tiles per 32

            with tc.tile_pool(name="cps", bufs=2, space="PSUM") as cps, \
                 tc.tile_pool(name="sps", bufs=2, space="PSUM") as sps, \
                 tc.tile_pool(name="dmp", bufs=2) as dmp:
                for it in range(NIT):
                    sl = m1acc[:, it * IT:(it + 1) * IT]
                    for jt in range(NJT):
                        js = slice(jt * 128, (jt + 1) * 128)
                        idx = jt * NIT + it
                        scopy = (jt * nsc) // NJT != ((jt + 1) * nsc) // NJT
                        # content: psG = lcx^T rx + lcc^T rc   (K = 512)
                        psG = cps.tile([128, IT], dt.float32, tag="psG",
                                       name=f"psG{idx}")
                        for m in range(2):
                            ms = slice(it * IT + m * 512, it * IT + (m + 1) * 512)
                            os = slice(m * 512, (m + 1) * 512)
                            mm(psG[:, os], LCX, RX, js, ms, True, False)
                            mm(psG[:, os], LCC, RC, js, ms, False, True)
                        # style: psS = ls^T rx   (K = 256)
                        psS = sps.tile([128, IT], dt.float32, tag="psS",
                                       name=f"psS{idx}")
                        for m in range(2):
                            ms = slice(it * IT + m * 512, it * IT + (m + 1) * 512)
                            os = slice(m * 512, (m + 1) * 512)
                            mm(psS[:, os], LS, RX, js, ms, True, True)
                        stage = dmp.tile([128, IT], dt.bfloat16, tag="stg",
                                         name=f"stg{idx}")
                        if scopy:
                            # offload the PSUM->bf16 staging copy to Scalar
                            # (issued before the abs so the DVE isn't blocked)
                            nc.scalar.activation(stage[:], psS[:], AF.Copy)
                        # content |.| accumulate on Scalar
                        dump = dmp.tile([128, IT], dt.bfloat16, tag="adump",
                                        name=f"adump{idx}")
                        nc.scalar.activation(
                            dump[:], psG[:], AF.Abs,
                            accum_out=csum_slots[:, idx:idx + 1])
                        # style: ONE fused DVE pass = bf16 staging copy of
                        # psS + clean per-j m2 max; then a cheap 2x-mode
                        # bf16 max folds the stage into the m1 running max.
                        if scopy:
                            nc.vector.reduce_max(m2slots[:, idx:idx + 1],
                                                 stage[:], axis=AX.X)
                        else:
                            nc.vector._custom_dve(
                                dve_op, out=stage[:], in0=psS[:], s1=-2.0,
                                imm2=1.0,
                                accum_out=m2slots[:, idx:idx + 1])
                        nc.vector.tensor_max(sl, sl, stage[:])
                    # stream out the finished half of m1acc
                    nc.sync.dma_start(o_m1[:, it * IT:(it + 1) * IT],
                                      m1acc[:, it * IT:(it + 1) * IT])
                nc.sync.dma_start(o_cs[:], csum_slots[:])
                nc.sync.dma_start(o_m2[:], m2slots[:])

    nc.finalize()
    return nc


def _get_nc():
    global _CACHED_NC
    if _CACHED_NC is None:
        import os
        _CACHED_NC = _build(repeat=int(os.environ.get("KREPEAT", "1")))
    return _CACHED_NC


_RUNNER = None


def _get_runner():
    """Compile the 8-core PJRT executable once; returns run(in_maps)->results.

    Mirrors concourse.bass2jax.run_bass_via_pjrt but caches the jitted
    executable so repeated kernel() calls only pay device execution.
    """
    global _RUNNER
    if _RUNNER is not None:
        return _RUNNER
    import jax
    import numpy as _np
    from jax.sharding import Mesh, PartitionSpec
    from jax.experimental.shard_map import shard_map
    from concourse import mybir, bass2jax
    from concourse.bass2jax import _bass_exec_p, partition_id_tensor

    bass2jax.install_neuronx_cc_hook()
    nc = _get_nc()
    partition_name = (nc.partition_id_tensor.name
                      if nc.partition_id_tensor else None)

    in_names, out_names, out_avals, zero_outs = [], [], [], []
    for alloc in nc.m.functions[0].allocations:
        if not isinstance(alloc, mybir.MemoryLocationSet):
            continue
        name = alloc.memorylocations[0].name
        if alloc.kind == "ExternalInput":
            if name != partition_name:
                in_names.append(name)
        elif alloc.kind == "ExternalOutput":
            out_names.append(name)
            shape = tuple(alloc.tensor_shape)
            dtype = mybir.dt.np(alloc.dtype)
            out_avals.append(jax.core.ShapedArray(shape, dtype))
            zero_outs.append(_np.zeros((NCORES * shape[0], *shape[1:]), dtype))
    n_params = len(in_names)
    n_outs = len(out_avals)
    all_names = list(in_names) + list(out_names)
    if partition_name is not None:
        all_names.append(partition_name)
    donate = tuple(range(n_params, n_params + n_outs))

    def _body(*args):
        operands = list(args)
        if partition_name is not None:
            operands.append(partition_id_tensor())
        return tuple(_bass_exec_p.bind(
            *operands,
            out_avals=tuple(out_avals),
            in_names=tuple(all_names),
            out_names=tuple(out_names),
            lowering_input_output_aliases=(),
            sim_require_finite=True,
            sim_require_nnan=True,
            nc=nc,
        ))

    devices = jax.devices()[:NCORES]
    mesh = Mesh(_np.asarray(devices), ("core",))
    sharded = jax.jit(
        shard_map(_body, mesh=mesh,
                  in_specs=(PartitionSpec("core"),) * (n_params + n_outs),
                  out_specs=(PartitionSpec("core"),) * n_outs,
                  check_rep=False),
        donate_argnums=donate, keep_unused=True,
    )

    def prepare(in_maps):
        """Stage concatenated inputs onto the devices once (for timing)."""
        from jax.sharding import NamedSharding
        sh = NamedSharding(mesh, PartitionSpec("core"))
        concat_in = [
            _np.concatenate([in_maps[c][nm] for c in range(NCORES)], axis=0)
            for nm in in_names
        ]
        return [jax.device_put(a, sh) for a in concat_in]

    def exec_prepared(staged):
        out_arrs = sharded(*staged, *zero_outs)
        jax.block_until_ready(out_arrs)
        return out_arrs

    def run(in_maps):
        concat_in = [
            _np.concatenate([in_maps[c][nm] for c in range(NCORES)], axis=0)
            for nm in in_names
        ]
        out_arrs = sharded(*concat_in, *zero_outs)
        jax.block_until_ready(out_arrs)
        return [
            {nm: _np.asarray(out_arrs[i]).reshape(NCORES, *out_avals[i].shape)[c]
             for i, nm in enumerate(out_names)}
            for c in range(NCORES)
        ]

    run.prepare = prepare
    run.exec_prepared = exec_prepared
    _RUNNER = run
    return run


def _pack(a):
    """[C, N] -> DoubleRow layout [128, 2, N]: channel c = r*128 + k."""
    n = a.shape[1]
    return np.ascontiguousarray(a.reshape(2, 128, n).transpose(1, 0, 2))


def _make_in_maps(x_feat, c_feat, s_feat):
    from concourse import mybir
    f8 = mybir.dt.np(mybir.dt.bfloat16 if _variant() == "bf16"
                     else mybir.dt.float8e4)
    x = np.asarray(x_feat, dtype=np.float32).reshape(B, C, HW)
    c = np.asarray(c_feat, dtype=np.float32).reshape(B, C, HW)
    s = np.asarray(s_feat, dtype=np.float32).reshape(B, C, HW)

    in_maps = [dict() for _ in range(NCORES)]
    for b in range(B):
        def hat(t):
            return t / np.sqrt((t * t).sum(axis=0, keepdims=True))
        hx, hc, hs = hat(x[b]), hat(c[b]), hat(s[b])

        def sinv_scaled(h):
            u = h.sum(axis=1)
            su = u @ h
            return HW / (HW - su)
        lx = _pack((hx * sinv_scaled(hx)[None, :]).astype(f8))
        lc = _pack((-hc * sinv_scaled(hc)[None, :]).astype(f8))
        lsb = _pack(hs.astype(f8))
        rxb = _pack(hx.astype(f8))
        rcb = _pack(hc.astype(f8))
        for ih in range(2):
            sl = slice(ih * IQ, (ih + 1) * IQ)
            in_maps[2 * b + ih] = {
                "lcx": lx, "lcc": lc, "ls": lsb,
                "rx": np.ascontiguousarray(rxb[:, :, sl]),
                "rc": np.ascontiguousarray(rcb[:, :, sl]),
            }
    return in_maps


def kernel(x_feat, c_feat, s_feat):
    outs = _get_runner()(_make_in_maps(x_feat, c_feat, s_feat))

    total = sum(float(r["csum"].sum()) for r in outs)
    content = total / (B * HW * HW)

    m1vals = 1.0 - np.concatenate(
        [r["m1acc"].astype(np.float32).max(axis=0) for r in outs])
    m1mean = float(m1vals.mean())
    m2mean = 0.0
    for b_ in range(B):
        m2a = outs[2 * b_]["m2part"].reshape(128, NJT, NIT).max(axis=2)
        m2b = outs[2 * b_ + 1]["m2part"].reshape(128, NJT, NIT).max(axis=2)
        mx = np.maximum(m2a, m2b)
        m2mean += float((1.0 - mx).mean())
    m2mean /= B
    style = max(m1mean, m2mean)

    return (np.float32(content), np.float32(style))
